# revision 1
# baseline (speedup 1.0000x reference)
"""Trainium2 Bass kernel for fused QKV projection + interleaved RoPE.

Problem: X[4, 4096, 2048] @ {Wq, Wk, Wv}[2048, 2048] -> reshape to heads
[B, S, 16, 128], apply interleaved RoPE to Q and K, return (Xq, Xk, Xv).

Sharding: data-parallel over tokens. The 4*4096 = 16384 token rows are
split into 8 contiguous shards of 2048 rows (core c gets batch c//2,
sequence half c%2). Every core holds the full Wq/Wk/Wv and computes all
2048 output features for its rows; RoPE is per-token elementwise so no
communication is needed.

Device kernel (identical SPMD program on all 8 cores):
  - X^T shard (cast to bf16 on host) stays resident in SBUF as 16
    per-row-chunk tiles; weights stream through double-buffered half-M
    tiles so each of the six (tensor, m-half) phases prefetches the next.
  - matmul out = lhsT.T @ rhs with lhsT = X^T tile [128k, 128r]
    (stationary) and rhs = W tile [128k, 512m] (moving), accumulating
    psum[128r, 1024m] fp32 over 16 k-chunks.
  - RoPE in 3 DVE ops on the psum tile: the interleaved pair swap is a
    reversed-stride access pattern, the rotation sign is pre-baked into
    the sin table on the host, and cos/sin broadcast across heads via
    zero-stride APs. V is copied back on the scalar engine.
"""

import numpy as np
import ml_dtypes

import concourse.bass as bass
import concourse.mybir as mybir
import concourse.tile as tile
from concourse import bacc
from concourse.bass import ds, ts
from concourse.bass_utils import run_bass_kernel_spmd

B, S, DIM, H = 4, 4096, 2048, 16
HD = DIM // H           # 128
N_CORES = 8
R = B * S // N_CORES    # 2048 token rows per core
P = 128

BF16 = mybir.dt.bfloat16
F32 = mybir.dt.float32


def build_nc(K=DIM, M=DIM, rows=R, hd=HD, mm_free=512, m_half=1024, loop_n=1,
             unroll=False):
    """Build the per-core Bass program.

    K: contraction dim, M: output feature dim, rows: token rows per core.
    loop_n > 1 wraps the body in a device-side For_i for benchmarking.
    """
    m_half = min(m_half, M)
    assert K % P == 0 and rows % P == 0 and M % m_half == 0
    assert m_half % mm_free == 0 and m_half % hd == 0
    KO = K // P           # k-chunks
    RC = rows // P        # token row chunks
    HALVES = M // m_half  # weight column phases per tensor
    MJ = m_half // mm_free
    NH = m_half // hd     # heads per column phase
    NI_SWEEP = 2          # rc's interleaved in the cold-start k-sweep
    J = hd // 2           # rotation pairs per head

    nc = bacc.Bacc(None, target_bir_lowering=False)

    # xt is host-permuted to [rc, p, ko, r] so each per-rc tile DMA reads
    # one contiguous 4 KB run per partition (strided 256 B gathers measured
    # 88 GB/s and pushed the first matmul out to ~14 us).
    xt = nc.dram_tensor("xt", [rows // P, P, K // P * P], BF16,
                        kind="ExternalInput")
    wq = nc.dram_tensor("wq", [K, M], BF16, kind="ExternalInput")
    wk = nc.dram_tensor("wk", [K, M], BF16, kind="ExternalInput")
    wv = nc.dram_tensor("wv", [K, M], BF16, kind="ExternalInput")
    cosf = nc.dram_tensor("cosf", [P, rows // P * hd], F32,
                          kind="ExternalInput")
    ssin = nc.dram_tensor("ssin", [P, rows // P * hd], F32,
                          kind="ExternalInput")
    q_out = nc.dram_tensor("q", [rows, M], F32, kind="ExternalOutput")
    k_out = nc.dram_tensor("k", [rows, M], F32, kind="ExternalOutput")
    v_out = nc.dram_tensor("v", [rows, M], F32, kind="ExternalOutput")

    xt_r = xt[:]
    cos_r = cosf[:]
    sin_r = ssin[:]

    with tile.TileContext(nc) as tc:
        with (
            tc.tile_pool(name="wpool", bufs=2 * (K // P)) as wpool,
            tc.tile_pool(name="xpool", bufs=RC) as xpool,
            tc.tile_pool(name="cpool", bufs=1) as cpool,
            tc.tile_pool(name="opool", bufs=4) as opool,
            tc.tile_pool(name="tpool", bufs=2) as tpool,
            tc.tile_pool(name="psum", bufs=4, space="PSUM") as pspool,
        ):
            def load_w_tiles(w_r, half, split_rings=False):
                # per-ko tiles so the first matmul only waits on 256 KB.
                # split_rings (cold start): odd-ko tiles go on the SP ring,
                # which is free once the small x0/x1 chunks are issued, so
                # both HWDGE rings deliver W and the first k-sweep never
                # starves.
                tiles = []
                for ko in range(KO):
                    w_sb = wpool.tile([P, m_half], BF16, tag="w")
                    eng = nc.sync if (split_rings and ko % 2) else nc.scalar
                    eng.dma_start(w_sb[:], w_r[:, ko, ts(half, m_half)])
                    tiles.append(w_sb)
                return tiles

            def lhsT_of(xt_tiles, rc, ko):
                xt = xt_tiles[rc]
                if isinstance(xt, list):  # ko-chunked tile list
                    per = KO // len(xt)
                    return xt[ko // per][:, ko % per]
                return xt[:, ko]

            def emit_phase(w_tiles, o_r, half, rope, xt_tiles, cos_sb, sin_sb,
                           pair0=False, split_last=False):
                start_rc = 0
                if pair0 and RC >= 3:
                    # The first k-sweep's W tiles stream in while the sweep
                    # runs; interleave the first NI_SWEEP rc's (psums live,
                    # same tiles) so each W tile feeds 2*NI_SWEEP matmuls and
                    # consumption matches the early-HBM delivery rate.
                    pss = [
                        pspool.tile([P, m_half], F32, tag="ps", name=f"ps_p{i}")
                        for i in range(NI_SWEEP)
                    ]
                    for ko in range(KO):
                        for rc, psx in enumerate(pss):
                            for mj in range(MJ):
                                nc.tensor.matmul(
                                    psx[:, ts(mj, mm_free)],
                                    lhsT_of(xt_tiles, rc, ko),
                                    w_tiles[ko][:, ts(mj, mm_free)],
                                    start=(ko == 0),
                                    stop=(ko == KO - 1),
                                )
                    for rc, psx in enumerate(pss):
                        finish_rc(psx, o_r, half, rc, rope, cos_sb, sin_sb)
                    start_rc = NI_SWEEP
                for rc in range(start_rc, RC):
                    psum = pspool.tile([P, m_half], F32, tag="ps")
                    for ko in range(KO):
                        for mj in range(MJ):
                            nc.tensor.matmul(
                                psum[:, ts(mj, mm_free)],
                                lhsT_of(xt_tiles, rc, ko),
                                w_tiles[ko][:, ts(mj, mm_free)],
                                start=(ko == 0),
                                stop=(ko == KO - 1),
                            )
                    if split_last and rc == RC - 1 and m_half % (4 * hd) == 0:
                        # pipeline the very last tile's finish+store so the
                        # kernel tail after the final matmul is short:
                        # 4 column chunks, copies alternating ACT/DVE and
                        # stores alternating the two HWDGE rings so the
                        # final chains run in parallel.
                        mc4 = m_half // 4
                        for cj in range(4):
                            finish_rc(psum, o_r, half, rc, rope, cos_sb,
                                      sin_sb, c0=cj * mc4, mc=mc4,
                                      fin_alt=(cj % 2 == 1))
                    else:
                        finish_rc(psum, o_r, half, rc, rope, cos_sb, sin_sb)

            def finish_rc(psum, o_r, half, rc, rope, cos_sb, sin_sb,
                          c0=0, mc=None, fin_alt=False):
                    mc = m_half if mc is None else mc
                    nh = mc // hd
                    ps = psum[:, ds(c0, mc)]
                    o_sb = opool.tile([P, mc], F32, tag="o")
                    if rope:
                        # o = x*cos + swap_pairs(x)*ssin; ssin sign-baked,
                        # the swap is a reversed-stride AP on the pair dim.
                        ps_hd = ps.rearrange("p (h d) -> p h d", d=hd)
                        ps_pr = ps.rearrange(
                            "p (h j two) -> p h j two", h=nh, two=2
                        )
                        cos_ts, crc = cos_sb
                        sin_ts, _ = sin_sb
                        c_t, s_t = cos_ts[rc // crc], sin_ts[rc // crc]
                        rcl = rc % crc
                        cos_b = c_t[:, rcl, None, :].to_broadcast([P, nh, hd])
                        sin_b = s_t[:, rcl].rearrange(
                            "p (j two) -> p j two", two=2
                        )[:, None, :, :].to_broadcast([P, nh, J, 2])

                        t_sb = tpool.tile([P, mc], F32, tag="t")
                        t_pr = t_sb[:].rearrange(
                            "p (h j two) -> p h j two", h=nh, two=2
                        )
                        o_hd = o_sb[:].rearrange("p (h d) -> p h d", d=hd)

                        nc.vector.tensor_tensor(
                            t_pr[:], ps_pr[:, :, :, ::-1], sin_b,
                            mybir.AluOpType.mult,
                        )
                        nc.vector.tensor_tensor(
                            o_hd, ps_hd, cos_b, mybir.AluOpType.mult,
                        )
                        nc.vector.tensor_tensor(
                            o_sb[:], o_sb[:], t_sb[:], mybir.AluOpType.add,
                        )
                    elif fin_alt:
                        nc.vector.tensor_copy(o_sb[:], ps)
                    else:
                        nc.scalar.copy(o_sb[:], ps)

                    # stores share the ACT HWDGE ring with the (small,
                    # interleaved) weight prefetches; activations + freqs
                    # own the SP ring so neither queue head-of-line blocks.
                    # fin_alt (tail chunks) stores on the idle SP ring.
                    st = nc.sync if fin_alt else nc.scalar
                    st.dma_start(
                        o_r[:, rc, ds(half * m_half + c0, mc)], o_sb[:])

            def body():
                # Cold-start ordering: the first matmuls need only x[0] and
                # the first W tiles, so issue those before everything else
                # (x on the SP HWDGE ring, W on ACT's). V-half0 first: no
                # cos/sin dependency during the contended cold start. V-half1
                # last: the kernel tail is copy+store, not the RoPE chain.
                def ph(w_dram, o_dram, rope, half):
                    w_r = w_dram[:].rearrange("(ko p) m -> p ko m", p=P)
                    o_r = o_dram[:].rearrange("(rc p) m -> p rc m", p=P)
                    return (w_r, o_r, half, rope)

                phases = [
                    ph(wv, v_out, False, 0),
                    ph(wq, q_out, True, 0),
                    ph(wq, q_out, True, 1) if HALVES > 1 else None,
                    ph(wk, k_out, True, 0),
                    ph(wk, k_out, True, 1) if HALVES > 1 else None,
                    ph(wv, v_out, False, 1) if HALVES > 1 else None,
                ]
                phases = [p for p in phases if p is not None]

                # Pre-warm the PE while the first DMAs are in flight: the
                # HAM clock gate starts at 1.2 GHz and needs ~3.4 us of
                # sustained matmul activity to release to 2.4 GHz. A block
                # of dummy matmuls on zeroed SBUF runs during the x0/W0
                # DMA wait so the real stream starts at full clock. The
                # first real accumulation starts with start=True, which
                # clears the garbage psum.
                N_WARM = 9
                if N_WARM and RC >= 3:
                    wl_sb = cpool.tile([P, P], BF16, tag="warm_l")
                    wr_sb = cpool.tile([P, mm_free], BF16, tag="warm_r")
                    nc.vector.memset(wl_sb[:], 0.0)
                    nc.vector.memset(wr_sb[:], 0.0)
                    warm_ps = pspool.tile([P, m_half], F32, tag="ps",
                                          name="ps_warm")
                    for _ in range(N_WARM):
                        nc.tensor.matmul(
                            warm_ps[:, ts(0, mm_free)], wl_sb[:], wr_sb[:],
                            start=True, stop=True,
                        )

                # Cold start: x0/x1 split into 4-ko chunks so the first
                # matmul waits on 128 KB, not 512 KB; phase-0 W spread over
                # BOTH HWDGE rings (odd ko on SP after the x chunks) so the
                # first k-sweep is fed at aggregate HBM rate.
                NI = min(2, RC)  # interleaved rc's in the first sweep
                XC = 4 if KO % 4 == 0 and RC >= 3 else 1
                per = KO // XC
                xch = [[] for _ in range(NI)]
                for c in range(XC):
                    for rc in range(NI):
                        x_sb = xpool.tile([P, per, P], BF16,
                                          tag=f"x{rc}_{c}", bufs=1)
                        nc.sync.dma_start(
                            x_sb[:].rearrange("p ko r -> p (ko r)"),
                            xt_r[rc, :, ds(c * per * P, per * P)])
                        xch[rc].append(x_sb)
                w_first = load_w_tiles(phases[0][0], phases[0][2],
                                       split_rings=True)

                xt_tiles = list(xch)
                for rc in range(NI, RC):
                    x_sb = xpool.tile([P, KO, P], BF16, tag="x")
                    nc.sync.dma_start(
                        x_sb[:].rearrange("p ko r -> p (ko r)"), xt_r[rc])
                    xt_tiles.append(x_sb)
                # cos/sin after the x stream: first needed by the first Q
                # finish (phase 1, >100 us in), so keep the 2 MB of tables
                # out of the contended cold-start HBM window entirely.
                CC = 4 if RC % 4 == 0 else 1
                crc = RC // CC
                cos_tiles, sin_tiles = [], []
                for c in range(CC):
                    c_sb = cpool.tile([P, crc, hd], F32, tag=f"cos{c}")
                    s_sb = cpool.tile([P, crc, hd], F32, tag=f"sin{c}")
                    nc.sync.dma_start(
                        c_sb[:].rearrange("p rc d -> p (rc d)"),
                        cos_r[:, ds(c * crc * hd, crc * hd)])
                    nc.sync.dma_start(
                        s_sb[:].rearrange("p rc d -> p (rc d)"),
                        sin_r[:, ds(c * crc * hd, crc * hd)])
                    cos_tiles.append(c_sb)
                    sin_tiles.append(s_sb)
                cos_sb = (cos_tiles, crc)
                sin_sb = (sin_tiles, crc)

                for i, (w_r, o_r, half, rope) in enumerate(phases):
                    w_tiles = w_first if i == 0 else load_w_tiles(w_r, half)
                    emit_phase(w_tiles, o_r, half, rope, xt_tiles, cos_sb,
                               sin_sb, pair0=(i == 0),
                               split_last=(i == len(phases) - 1))

            if loop_n == 1:
                body()
            elif unroll:
                for _ in range(loop_n):
                    body()
            else:
                with tc.For_i(0, loop_n, 1):
                    body()

    nc.compile()
    return nc


_NC_CACHE = {}


def _get_nc():
    if "nc" not in _NC_CACHE:
        _NC_CACHE["nc"] = build_nc()
    return _NC_CACHE["nc"]


def prepare_in_maps(X, freqs_cos, freqs_sin, Wq, Wk, Wv):
    X = np.asarray(X, dtype=np.float32)
    freqs_cos = np.asarray(freqs_cos, dtype=np.float32)
    freqs_sin = np.asarray(freqs_sin, dtype=np.float32)

    Xf = X.reshape(B * S, DIM)
    Xb = Xf.astype(ml_dtypes.bfloat16)
    wq_b = np.asarray(Wq, dtype=np.float32).astype(ml_dtypes.bfloat16)
    wk_b = np.asarray(Wk, dtype=np.float32).astype(ml_dtypes.bfloat16)
    wv_b = np.asarray(Wv, dtype=np.float32).astype(ml_dtypes.bfloat16)

    # Rotation sign baked into sin: out[2i] = x[2i]c - x[2i+1]s,
    # out[2i+1] = x[2i+1]c + x[2i]s.
    ssin_full = freqs_sin.copy()
    ssin_full[:, 0::2] *= -1.0

    in_maps = []
    RC = R // 128
    KO = DIM // 128
    for c in range(N_CORES):
        rows = slice(c * R, (c + 1) * R)
        s0 = (c % 2) * R  # sequence offset of this shard (R == S // 2)
        # [rc, p, ko, r]: per-rc-tile DMA reads 4 KB contiguous per partition
        xt_c = np.ascontiguousarray(
            Xb[rows].reshape(RC, 128, KO, 128).transpose(0, 3, 2, 1)
        ).reshape(RC, 128, KO * 128)
        # cos/sin as [p, rc, d] so the tile load is partition-contiguous
        cos_c = np.ascontiguousarray(
            freqs_cos[s0:s0 + R].reshape(RC, 128, HD).transpose(1, 0, 2)
        ).reshape(128, RC * HD)
        sin_c = np.ascontiguousarray(
            ssin_full[s0:s0 + R].reshape(RC, 128, HD).transpose(1, 0, 2)
        ).reshape(128, RC * HD)
        in_maps.append({
            "xt": xt_c,
            "wq": wq_b,
            "wk": wk_b,
            "wv": wv_b,
            "cosf": cos_c,
            "ssin": sin_c,
        })
    return in_maps


def assemble_outputs(results):
    Xq = np.empty((B * S, H, HD), dtype=np.float32)
    Xk = np.empty((B * S, H, HD), dtype=np.float32)
    Xv = np.empty((B * S, H, HD), dtype=np.float32)
    for c in range(N_CORES):
        rows = slice(c * R, (c + 1) * R)
        Xq[rows] = results[c]["q"].reshape(R, H, HD)
        Xk[rows] = results[c]["k"].reshape(R, H, HD)
        Xv[rows] = results[c]["v"].reshape(R, H, HD)

    return (
        Xq.reshape(B, S, H, HD),
        Xk.reshape(B, S, H, HD),
        Xv.reshape(B, S, H, HD),
    )


def kernel(X, freqs_cos, freqs_sin, attention_mask, Wq, Wk, Wv):
    in_maps = prepare_in_maps(X, freqs_cos, freqs_sin, Wq, Wk, Wv)
    nc = _get_nc()
    res = run_bass_kernel_spmd(nc, in_maps, list(range(N_CORES)))
    return assemble_outputs(res.results)

